# revision 21
# baseline (speedup 1.0000x reference)
"""Concatenation (additive/Bahdanau-style) attention Trainium2 kernel.

Math (per batch b):
    f = x @ W1[:H]          # [S, A]
    g = x @ W1[H:]          # [S, A]
    scores[i, j] = w2 . tanh(f[i] + g[j] + b1) + b2
    e = exp(scores) * (j < i)
    out[i] = sum_j e[i, j] x[j] / (sum_j e[i, j] + 1e-10)

Sharding: data-parallel over batch, one batch element per NeuronCore (B=8).
Everything stays on-chip; the [S, S, A] pairwise tensor never exists in HBM.

Per-core layout strategy:
  - j-block = 8 consecutive j values; partitions of the tanh tile hold
    (j8, a) pairs: p = 8*16 grid = j8*16 + a  (8 j's x 16 hidden units).
  - FB[p, i] = f[i, p%16] replicated 8x on partitions (one PE matmul with a
    host-replicated W1a).
  - G[p, jb] = g[8*jb + j8, a] + b1[a] with p = j8*16+a  (8 strided matmuls).
  - U[p, i] = FB[p, i] + G[p, jb]  (DVE tensor_scalar add, per-partition
    scalar) for the exact triangular range i >= 8*jb, then one big ACT tanh
    per group of 8 j-blocks (amortizes ACT fixed overhead).
  - scores via PE matmul with block-diag W2BD[p, m] = (p//16==m) * w2[p%16]
    contracting all 128 partitions -> [8 j, i] rows in PSUM; 16 j-blocks
    fill a [128, Lg] PSUM supertile (j = 128*g + p).
  - one ACT exp (bias=b2) per supertile PSUM -> SBUF e-tile; strictly-upper
    [128,128] mask on the diagonal chunk enforces j < i.
  - out: for each 128-row i-block, accumulate matmuls over supertiles g<=ib:
    lhsT = e_g[:, i-cols] (K=j), rhs = x_aug (x with a ones column) so the
    softmax denominator falls out of the same matmuls; then reciprocal+scale.
"""

import numpy as np

import concourse.bass as bass
import concourse.tile as tile
from concourse import bacc, mybir
from concourse.bass_utils import run_bass_kernel_spmd

B, S, H, A = 8, 1024, 128, 16
NCORES = 8
XAUG_W = H + 4  # x plus a ones column, padded to 132 floats (528 B)
NBLK = S // 8  # 128 j-blocks of 8

FT = mybir.ActivationFunctionType
F32 = mybir.dt.float32
BF16 = mybir.dt.float16  # fp16: same 1 col/cycle as bf16, 8x the mantissa

# Score-matmul dtype knob: bf16 streams 1 col/cycle (vs 4 for float32).
# (float32r also streams 1 col/cycle but requires dst partition 0 and is
# no more precise than bf16 on TRN2.)
SCORE_BF16 = True


def _build_nc():
    nc = bacc.Bacc(None)

    xaug_d = nc.declare_dram_parameter("x_aug", [S, XAUG_W], F32, isOutput=False)
    xT_d = nc.declare_dram_parameter("xT", [H, S], F32, isOutput=False)
    w1ra_d = nc.declare_dram_parameter("W1repA", [H, 128], F32, isOutput=False)
    w1b_d = nc.declare_dram_parameter("W1b32", [H, 2, 32], F32, isOutput=False)
    b1r_d = nc.declare_dram_parameter("b1rep", [128, 1], F32, isOutput=False)
    w2bd_dt = BF16 if SCORE_BF16 else F32
    w2bd_d = nc.declare_dram_parameter("W2BD32", [128, 4, 32], w2bd_dt, isOutput=False)
    mask_d = nc.declare_dram_parameter("SUmask", [128, 128], F32, isOutput=False)
    b2r_d = nc.declare_dram_parameter("b2rep", [128, 1], F32, isOutput=False)
    out_d = nc.declare_dram_parameter("out", [S, H], F32, isOutput=True)

    with tile.TileContext(nc) as tc:
        with (
            tc.tile_pool(name="consts", bufs=1) as consts,
            tc.tile_pool(name="u", bufs=2) as upool,
            tc.tile_pool(name="e", bufs=1) as epool,
            tc.tile_pool(name="o", bufs=3) as opool,
            tc.tile_pool(name="psb", bufs=2, space="PSUM") as ps_big,
            tc.tile_pool(name="pss", bufs=2, space="PSUM") as ps_small,
        ):
            # ---- load inputs ----
            xT = consts.tile([H, S], F32)
            nc.sync.dma_start(out=xT, in_=xT_d[:, :])
            xaug = consts.tile([128, 8, XAUG_W], F32)
            for g in range(8):
                nc.sync.dma_start(
                    out=xaug[:, g, :], in_=xaug_d[g * 128 : (g + 1) * 128, :]
                )
            w1ra = consts.tile([H, 128], F32)
            nc.sync.dma_start(out=w1ra, in_=w1ra_d[:, :])
            w1b32 = consts.tile([H, 2, 32], F32)
            nc.sync.dma_start(out=w1b32, in_=w1b_d[:, :, :])
            b1r = consts.tile([128, 1], F32)
            nc.sync.dma_start(out=b1r, in_=b1r_d[:, :])
            w2bd32 = consts.tile([128, 4, 32], w2bd_dt)
            nc.sync.dma_start(out=w2bd32, in_=w2bd_d[:, :, :])
            mask = consts.tile([128, 128], F32)
            nc.sync.dma_start(out=mask, in_=mask_d[:, :])
            b2r = consts.tile([128, 1], F32)
            nc.sync.dma_start(out=b2r, in_=b2r_d[:, :])

            # ---- FB[p, i] = sum_h W1repA[h, p] * xT[h, i] ----
            FB = consts.tile([128, S], F32)
            fbp = ps_big.tile([128, S], F32, tag="big")
            for c in range(2):
                sl = slice(c * 512, (c + 1) * 512)
                nc.tensor.matmul(
                    out=fbp[:, sl],
                    lhsT=w1ra[:, :],
                    rhs=xT[:, sl],
                    start=True,
                    stop=True,
                )
            nc.vector.tensor_copy(out=FB, in_=fbp)

            # ---- G[j8*16+a, jb] = sum_h W1b[h, a] * xT[h, 8*jb+j8] + b1[a] ----
            G = consts.tile([128, NBLK], F32)
            gp = ps_small.tile([128, NBLK], F32, tag="small")
            xTg = xT[:, :].rearrange("h (j e) -> h j e", e=8)
            for q in range(4):
                for r in range(2):
                    nc.tensor.matmul(
                        out=gp[32 * q : 32 * (q + 1), :],
                        lhsT=w1b32[:, r, :],
                        rhs=xTg[:, :, 2 * q + r],
                        start=(r == 0),
                        stop=(r == 1),
                        tile_position=(0, 32 * q),
                    )
            nc.vector.tensor_scalar_add(out=G, in0=gp, scalar1=b1r[:, :])

            # ---- main loop: supertiles of 16 j-blocks (128 j's) ----
            e_tiles = []
            for g in range(8):
                Lg = S - 128 * g  # psum supertile covers columns i in [128g, S)
                ps = ps_big.tile([128, Lg], F32, tag="big")
                # pre-zero the ragged diagonal chunk (cols not written by
                # matmuls there stay 0 -> exp -> masked to 0)
                nc.vector.memset(ps[:, 0:128], 0.0)
                for half in range(2):
                    jbs = [16 * g + 8 * half + k for k in range(8)]
                    offs = []
                    flat = 0
                    for jb in jbs:
                        offs.append(flat)
                        flat += S - 8 * jb
                    U = upool.tile([128, flat], F32, tag="u")
                    for jb, o in zip(jbs, offs):
                        Lb = S - 8 * jb
                        nc.vector.tensor_scalar_add(
                            out=U[:, o : o + Lb],
                            in0=FB[:, 8 * jb : S],
                            scalar1=G[:, jb : jb + 1],
                        )
                    # tanh output in bf16 so score matmuls stream 1 col/cycle
                    if SCORE_BF16:
                        TT = upool.tile([128, flat], BF16, tag="tt")
                        nc.scalar.activation(out=TT[:, :], in_=U[:, :], func=FT.Tanh)
                    else:
                        TT = U
                        nc.scalar.activation(out=U[:, :], in_=U[:, :], func=FT.Tanh)
                    # PE output partitions must be 32-aligned: accumulate 4
                    # j-blocks into each 32-row group with 4 shifted
                    # block-diag lhsT variants (r=0 zero-inits the group).
                    for sub in range(2):
                        q = 2 * half + sub
                        for r in range(4):
                            jb = jbs[4 * sub + r]
                            o = offs[4 * sub + r]
                            rel0 = 8 * jb - 128 * g  # == 32q + 8r
                            bounds = (
                                [rel0] + [b for b in (512,) if rel0 < b < Lg] + [Lg]
                            )
                            for c0, c1 in zip(bounds[:-1], bounds[1:]):
                                rhs_ap = TT[:, o + (c0 - rel0) : o + (c1 - rel0)]
                                lhs_ap = w2bd32[:, r, :]
                                nc.tensor.matmul(
                                    out=ps[32 * q : 32 * (q + 1), c0:c1],
                                    lhsT=lhs_ap,
                                    rhs=rhs_ap,
                                    start=(r == 0),
                                    stop=(r == 3),
                                    tile_position=(0, 32 * q),
                                )
                e = epool.tile([128, Lg], F32, tag=f"e{g}")
                nc.scalar.activation(
                    out=e[:, :], in_=ps[:, :], func=FT.Exp, bias=b2r[:, :], scale=1.0
                )
                nc.vector.tensor_mul(e[:, 0:128], e[:, 0:128], mask[:, :])
                e_tiles.append(e)

            # ---- out[i, :] = (sum_j e[j, i] * x_aug[j]) ; denom in col H ----
            for ib in range(8):
                po = ps_small.tile([128, XAUG_W], F32, tag="small")
                for g in range(ib + 1):
                    col0 = 128 * (ib - g)
                    nc.tensor.matmul(
                        out=po[:, :],
                        lhsT=e_tiles[g][:, col0 : col0 + 128],
                        rhs=xaug[:, g, :],
                        start=(g == 0),
                        stop=(g == ib),
                    )
                rec = opool.tile([128, 1], F32, tag="rec")
                nc.vector.tensor_scalar_add(
                    out=rec, in0=po[:, H : H + 1], scalar1=1e-10
                )
                nc.vector.reciprocal(out=rec, in_=rec)
                osb = opool.tile([128, H], F32, tag="osb")
                nc.vector.tensor_scalar_mul(out=osb, in0=po[:, 0:H], scalar1=rec)
                nc.sync.dma_start(
                    out=out_d[ib * 128 : (ib + 1) * 128, :], in_=osb
                )

    nc.compile()
    return nc


_NC_CACHE = None


def _get_nc():
    global _NC_CACHE
    if _NC_CACHE is None:
        _NC_CACHE = _build_nc()
    return _NC_CACHE


def _host_prep(x, W1, b1, w2, b2):
    """Build the per-core input maps (all small derived tensors + shards)."""
    x = np.asarray(x, dtype=np.float32)
    W1 = np.asarray(W1, dtype=np.float32)
    b1 = np.asarray(b1, dtype=np.float32).reshape(-1)
    w2 = np.asarray(w2, dtype=np.float32).reshape(-1)
    b2 = np.asarray(b2, dtype=np.float32).reshape(-1)

    p = np.arange(128)
    W1repA = np.ascontiguousarray(W1[:H][:, p % A])  # [H, 128]
    # W1b32[h, r, m] places g-matmul outputs for j8 = 2q+r at rows 16r+a
    W1b32 = np.zeros((H, 2, 32), dtype=np.float32)
    for r in range(2):
        W1b32[:, r, 16 * r : 16 * r + A] = W1[H:]
    b1rep = np.ascontiguousarray(b1[p % A].reshape(128, 1))
    # W2BD32[p, r, m] = w2[a] iff m == 8r + j8, with p = j8*16 + a
    W2BD32 = np.zeros((128, 4, 32), dtype=np.float32)
    for r in range(4):
        W2BD32[p, r, 8 * r + p // A] = w2[p % A]
    if SCORE_BF16:
        W2BD32 = W2BD32.astype(np.float16)
    SUmask = (p[:, None] < p[None, :]).astype(np.float32)  # strictly upper
    b2rep = np.full((128, 1), b2[0], dtype=np.float32)

    shared = {
        "W1repA": W1repA,
        "W1b32": W1b32,
        "b1rep": b1rep,
        "W2BD32": W2BD32,
        "SUmask": SUmask,
        "b2rep": b2rep,
    }
    in_maps = []
    for c in range(NCORES):
        xb = x[c]  # [S, H]
        x_aug = np.zeros((S, XAUG_W), dtype=np.float32)
        x_aug[:, :H] = xb
        x_aug[:, H] = 1.0
        m = dict(shared)
        m["x_aug"] = x_aug
        m["xT"] = np.ascontiguousarray(xb.T)
        in_maps.append(m)
    return in_maps


def kernel(x, W1, b1, w2, b2, _trace=False):
    nc = _get_nc()
    in_maps = _host_prep(x, W1, b1, w2, b2)
    res = run_bass_kernel_spmd(nc, in_maps, list(range(NCORES)), trace=_trace)
    out = np.stack([np.asarray(res.results[c]["out"]) for c in range(NCORES)])
    if _trace:
        kernel.last_exec_time_ns = res.exec_time_ns
        kernel.last_profile = res.profile_json
    return out


# revision 22
# speedup vs baseline: 1.0645x; 1.0645x over previous
"""Concatenation (additive/Bahdanau-style) attention Trainium2 kernel.

Math (per batch b):
    f = x @ W1[:H]          # [S, A]
    g = x @ W1[H:]          # [S, A]
    scores[i, j] = w2 . tanh(f[i] + g[j] + b1) + b2
    e = exp(scores) * (j < i)
    out[i] = sum_j e[i, j] x[j] / (sum_j e[i, j] + 1e-10)

Sharding: data-parallel over batch, one batch element per NeuronCore (B=8).
Everything stays on-chip; the [S, S, A] pairwise tensor never exists in HBM.

Per-core layout strategy:
  - j-block = 8 consecutive j values; partitions of the tanh tile hold
    (j8, a) pairs: p = 8*16 grid = j8*16 + a  (8 j's x 16 hidden units).
  - FB[p, i] = f[i, p%16] replicated 8x on partitions (one PE matmul with a
    host-replicated W1a).
  - G[p, jb] = g[8*jb + j8, a] + b1[a] with p = j8*16+a  (8 strided matmuls).
  - U[p, i] = FB[p, i] + G[p, jb]  (DVE tensor_scalar add, per-partition
    scalar) for the exact triangular range i >= 8*jb, then one big ACT tanh
    per group of 8 j-blocks (amortizes ACT fixed overhead).
  - scores via PE matmul with block-diag W2BD[p, m] = (p//16==m) * w2[p%16]
    contracting all 128 partitions -> [8 j, i] rows in PSUM; 16 j-blocks
    fill a [128, Lg] PSUM supertile (j = 128*g + p).
  - one ACT exp (bias=b2) per supertile PSUM -> SBUF e-tile; strictly-upper
    [128,128] mask on the diagonal chunk enforces j < i.
  - out: for each 128-row i-block, accumulate matmuls over supertiles g<=ib:
    lhsT = e_g[:, i-cols] (K=j), rhs = x_aug (x with a ones column) so the
    softmax denominator falls out of the same matmuls; then reciprocal+scale.
"""

import numpy as np

import concourse.bass as bass
import concourse.tile as tile
from concourse import bacc, mybir
from concourse.bass_utils import run_bass_kernel_spmd

B, S, H, A = 8, 1024, 128, 16
NCORES = 8
XAUG_W = H + 4  # x plus a ones column, padded to 132 floats (528 B)
NBLK = S // 8  # 128 j-blocks of 8

FT = mybir.ActivationFunctionType
F32 = mybir.dt.float32
BF16 = mybir.dt.float16  # fp16: same 1 col/cycle as bf16, 8x the mantissa

# Score-matmul dtype knob: bf16 streams 1 col/cycle (vs 4 for float32).
# (float32r also streams 1 col/cycle but requires dst partition 0 and is
# no more precise than bf16 on TRN2.)
SCORE_BF16 = True


def _build_nc():
    nc = bacc.Bacc(None)

    xaug_d = nc.declare_dram_parameter("x_aug", [S, XAUG_W], F32, isOutput=False)
    xT_d = nc.declare_dram_parameter("xT", [H, S], F32, isOutput=False)
    w1ra_d = nc.declare_dram_parameter("W1repA", [H, 128], F32, isOutput=False)
    w1b_d = nc.declare_dram_parameter("W1b32", [H, 2, 32], F32, isOutput=False)
    b1r_d = nc.declare_dram_parameter("b1rep", [128, 1], F32, isOutput=False)
    w2bd_dt = BF16 if SCORE_BF16 else F32
    w2bd_d = nc.declare_dram_parameter("W2BDpad", [128, 248], w2bd_dt, isOutput=False)
    mask_d = nc.declare_dram_parameter("SUmask", [128, 128], F32, isOutput=False)
    b2r_d = nc.declare_dram_parameter("b2rep", [128, 1], F32, isOutput=False)
    out_d = nc.declare_dram_parameter("out", [S, H], F32, isOutput=True)

    with tile.TileContext(nc) as tc:
        with (
            tc.tile_pool(name="consts", bufs=1) as consts,
            tc.tile_pool(name="u", bufs=2) as upool,
            tc.tile_pool(name="e", bufs=1) as epool,
            tc.tile_pool(name="o", bufs=3) as opool,
            tc.tile_pool(name="psb", bufs=2, space="PSUM") as ps_big,
            tc.tile_pool(name="pss", bufs=2, space="PSUM") as ps_small,
        ):
            # ---- load inputs ----
            xT = consts.tile([H, S], F32)
            nc.sync.dma_start(out=xT, in_=xT_d[:, :])
            xaug = consts.tile([128, 8, XAUG_W], F32)
            for g in range(8):
                nc.sync.dma_start(
                    out=xaug[:, g, :], in_=xaug_d[g * 128 : (g + 1) * 128, :]
                )
            w1ra = consts.tile([H, 128], F32)
            nc.sync.dma_start(out=w1ra, in_=w1ra_d[:, :])
            w1b32 = consts.tile([H, 2, 32], F32)
            nc.sync.dma_start(out=w1b32, in_=w1b_d[:, :, :])
            b1r = consts.tile([128, 1], F32)
            nc.sync.dma_start(out=b1r, in_=b1r_d[:, :])
            w2pad = consts.tile([128, 248], w2bd_dt)
            nc.sync.dma_start(out=w2pad, in_=w2bd_d[:, :])
            mask = consts.tile([128, 128], F32)
            nc.sync.dma_start(out=mask, in_=mask_d[:, :])
            b2r = consts.tile([128, 1], F32)
            nc.sync.dma_start(out=b2r, in_=b2r_d[:, :])

            # ---- FB[p, i] = sum_h W1repA[h, p] * xT[h, i] ----
            FB = consts.tile([128, S], BF16)
            fbp = ps_big.tile([128, S], F32, tag="big")
            for c in range(2):
                sl = slice(c * 512, (c + 1) * 512)
                nc.tensor.matmul(
                    out=fbp[:, sl],
                    lhsT=w1ra[:, :],
                    rhs=xT[:, sl],
                    start=True,
                    stop=True,
                )
            nc.vector.tensor_copy(out=FB, in_=fbp)

            # ---- G[j8*16+a, jb] = sum_h W1b[h, a] * xT[h, 8*jb+j8] + b1[a] ----
            G = consts.tile([128, NBLK], F32)
            gp = ps_small.tile([128, NBLK], F32, tag="small")
            xTg = xT[:, :].rearrange("h (j e) -> h j e", e=8)
            for q in range(4):
                for r in range(2):
                    nc.tensor.matmul(
                        out=gp[32 * q : 32 * (q + 1), :],
                        lhsT=w1b32[:, r, :],
                        rhs=xTg[:, :, 2 * q + r],
                        start=(r == 0),
                        stop=(r == 1),
                        tile_position=(0, 32 * q),
                    )
            nc.vector.tensor_scalar_add(out=G, in0=gp, scalar1=b1r[:, :])

            # ---- main loop: supertiles of 16 j-blocks (128 j's) ----
            e_tiles = []
            for g in range(8):
                Lg = S - 128 * g  # psum supertile covers columns i in [128g, S)
                ps = ps_big.tile([128, Lg], F32, tag="big")
                for half in range(2):
                    jbs = [16 * g + 8 * half + k for k in range(8)]
                    offs = []
                    flat = 0
                    for jb in jbs:
                        offs.append(flat)
                        flat += S - 8 * jb
                    U = upool.tile([128, flat], BF16, tag="u")
                    for jb, o in zip(jbs, offs):
                        Lb = S - 8 * jb
                        nc.vector.tensor_scalar_add(
                            out=U[:, o : o + Lb],
                            in0=FB[:, 8 * jb : S],
                            scalar1=G[:, jb : jb + 1],
                        )
                    # tanh output fp16 so score matmuls stream 1 col/cycle
                    TT = upool.tile([128, flat], BF16, tag="tt")
                    nc.scalar.activation(out=TT[:, :], in_=U[:, :], func=FT.Tanh)
                    # score matmuls: M=128 sliding-window block-diag weights
                    # (full-width weights enable fast-weight-load; out base
                    # partition always 0; k=0 zero-inits the whole supertile
                    # because its weight columns outside block 0 are zero)
                    for k8, (jb, o) in enumerate(zip(jbs, offs)):
                        k = jb - 16 * g  # block index within supertile
                        rel0 = 8 * jb - 128 * g  # == 8k
                        lhs_ap = w2pad[:, 120 - 8 * k : 248 - 8 * k]
                        bounds = (
                            [rel0] + [b for b in (512,) if rel0 < b < Lg] + [Lg]
                        )
                        for c0, c1 in zip(bounds[:-1], bounds[1:]):
                            nc.tensor.matmul(
                                out=ps[:, c0:c1],
                                lhsT=lhs_ap,
                                rhs=TT[:, o + (c0 - rel0) : o + (c1 - rel0)],
                                start=(k == 0),
                                stop=(k == 15),
                            )
                e = epool.tile([128, Lg], F32, tag=f"e{g}")
                nc.scalar.activation(
                    out=e[:, :], in_=ps[:, :], func=FT.Exp, bias=b2r[:, :], scale=1.0
                )
                nc.vector.tensor_mul(e[:, 0:128], e[:, 0:128], mask[:, :])
                e_tiles.append(e)

            # ---- out[i, :] = (sum_j e[j, i] * x_aug[j]) ; denom in col H ----
            for ib in range(8):
                po = ps_small.tile([128, XAUG_W], F32, tag="small")
                for g in range(ib + 1):
                    col0 = 128 * (ib - g)
                    nc.tensor.matmul(
                        out=po[:, :],
                        lhsT=e_tiles[g][:, col0 : col0 + 128],
                        rhs=xaug[:, g, :],
                        start=(g == 0),
                        stop=(g == ib),
                    )
                rec = opool.tile([128, 1], F32, tag="rec")
                nc.vector.tensor_scalar_add(
                    out=rec, in0=po[:, H : H + 1], scalar1=1e-10
                )
                nc.vector.reciprocal(out=rec, in_=rec)
                osb = opool.tile([128, H], F32, tag="osb")
                nc.vector.tensor_scalar_mul(out=osb, in0=po[:, 0:H], scalar1=rec)
                nc.sync.dma_start(
                    out=out_d[ib * 128 : (ib + 1) * 128, :], in_=osb
                )

    nc.compile()
    return nc


_NC_CACHE = None


def _get_nc():
    global _NC_CACHE
    if _NC_CACHE is None:
        _NC_CACHE = _build_nc()
    return _NC_CACHE


def _host_prep(x, W1, b1, w2, b2):
    """Build the per-core input maps (all small derived tensors + shards)."""
    x = np.asarray(x, dtype=np.float32)
    W1 = np.asarray(W1, dtype=np.float32)
    b1 = np.asarray(b1, dtype=np.float32).reshape(-1)
    w2 = np.asarray(w2, dtype=np.float32).reshape(-1)
    b2 = np.asarray(b2, dtype=np.float32).reshape(-1)

    p = np.arange(128)
    W1repA = np.ascontiguousarray(W1[:H][:, p % A])  # [H, 128]
    # W1b32[h, r, m] places g-matmul outputs for j8 = 2q+r at rows 16r+a
    W1b32 = np.zeros((H, 2, 32), dtype=np.float32)
    for r in range(2):
        W1b32[:, r, 16 * r : 16 * r + A] = W1[H:]
    b1rep = np.ascontiguousarray(b1[p % A].reshape(128, 1))
    # sliding-window block-diag weights: W2BDpad[p, 120 + j8] = w2[a]
    # (lhsT for block k is W2BDpad[:, 120-8k : 248-8k])
    W2BDpad = np.zeros((128, 248), dtype=np.float32)
    W2BDpad[p, 120 + p // A] = w2[p % A]
    if SCORE_BF16:
        W2BDpad = W2BDpad.astype(np.float16)
    SUmask = (p[:, None] < p[None, :]).astype(np.float32)  # strictly upper
    b2rep = np.full((128, 1), b2[0], dtype=np.float32)

    shared = {
        "W1repA": W1repA,
        "W1b32": W1b32,
        "b1rep": b1rep,
        "W2BDpad": W2BDpad,
        "SUmask": SUmask,
        "b2rep": b2rep,
    }
    in_maps = []
    for c in range(NCORES):
        xb = x[c]  # [S, H]
        x_aug = np.zeros((S, XAUG_W), dtype=np.float32)
        x_aug[:, :H] = xb
        x_aug[:, H] = 1.0
        m = dict(shared)
        m["x_aug"] = x_aug
        m["xT"] = np.ascontiguousarray(xb.T)
        in_maps.append(m)
    return in_maps


def kernel(x, W1, b1, w2, b2, _trace=False):
    nc = _get_nc()
    in_maps = _host_prep(x, W1, b1, w2, b2)
    res = run_bass_kernel_spmd(nc, in_maps, list(range(NCORES)), trace=_trace)
    out = np.stack([np.asarray(res.results[c]["out"]) for c in range(NCORES)])
    if _trace:
        kernel.last_exec_time_ns = res.exec_time_ns
        kernel.last_profile = res.profile_json
    return out


# revision 26
# speedup vs baseline: 1.0890x; 1.0230x over previous
"""Concatenation (additive/Bahdanau-style) attention Trainium2 kernel.

Math (per batch b):
    f = x @ W1[:H]          # [S, A]
    g = x @ W1[H:]          # [S, A]
    scores[i, j] = w2 . tanh(f[i] + g[j] + b1) + b2
    e = exp(scores) * (j < i)
    out[i] = sum_j e[i, j] x[j] / (sum_j e[i, j] + 1e-10)

Sharding: data-parallel over batch, one batch element per NeuronCore (B=8).
Everything stays on-chip; the [S, S, A] pairwise tensor never exists in HBM.

Per-core layout strategy:
  - j-block = 8 consecutive j values; partitions of the tanh tile hold
    (j8, a) pairs: p = 8*16 grid = j8*16 + a  (8 j's x 16 hidden units).
  - FB[p, i] = f[i, p%16] replicated 8x on partitions (one PE matmul with a
    host-replicated W1a).
  - G[p, jb] = g[8*jb + j8, a] + b1[a] with p = j8*16+a  (8 strided matmuls).
  - U[p, i] = FB[p, i] + G[p, jb]  (DVE tensor_scalar add, per-partition
    scalar) for the exact triangular range i >= 8*jb, then one big ACT tanh
    per group of 8 j-blocks (amortizes ACT fixed overhead).
  - scores via PE matmul with block-diag W2BD[p, m] = (p//16==m) * w2[p%16]
    contracting all 128 partitions -> [8 j, i] rows in PSUM; 16 j-blocks
    fill a [128, Lg] PSUM supertile (j = 128*g + p).
  - one ACT exp (bias=b2) per supertile PSUM -> SBUF e-tile; strictly-upper
    [128,128] mask on the diagonal chunk enforces j < i.
  - out: for each 128-row i-block, accumulate matmuls over supertiles g<=ib:
    lhsT = e_g[:, i-cols] (K=j), rhs = x_aug (x with a ones column) so the
    softmax denominator falls out of the same matmuls; then reciprocal+scale.
"""

import numpy as np

import concourse.bass as bass
import concourse.tile as tile
from concourse import bacc, mybir
from concourse.bass_utils import run_bass_kernel_spmd

B, S, H, A = 8, 1024, 128, 16
NCORES = 8
XAUG_W = H + 4  # x plus a ones column, padded to 132 floats (528 B)
NBLK = S // 8  # 128 j-blocks of 8

FT = mybir.ActivationFunctionType
F32 = mybir.dt.float32
BF16 = mybir.dt.float16  # fp16: same 1 col/cycle as bf16, 8x the mantissa

# Score-matmul dtype knob: bf16 streams 1 col/cycle (vs 4 for float32).
# (float32r also streams 1 col/cycle but requires dst partition 0 and is
# no more precise than bf16 on TRN2.)
SCORE_BF16 = True


def _build_nc():
    nc = bacc.Bacc(None)

    xaug_d = nc.declare_dram_parameter("x_aug", [S, XAUG_W], F32, isOutput=False)
    xT_d = nc.declare_dram_parameter("xT", [H, S], F32, isOutput=False)
    w1ra_d = nc.declare_dram_parameter("W1repA", [H, 128], F32, isOutput=False)
    w1b_d = nc.declare_dram_parameter("W1b32", [H, 2, 32], F32, isOutput=False)
    b1r_d = nc.declare_dram_parameter("b1rep", [128, 1], F32, isOutput=False)
    w2bd_dt = BF16 if SCORE_BF16 else F32
    w2bd_d = nc.declare_dram_parameter("W2BDpad", [128, 248], w2bd_dt, isOutput=False)
    mask_d = nc.declare_dram_parameter("SUmask", [128, 128], F32, isOutput=False)
    b2r_d = nc.declare_dram_parameter("b2rep", [128, 1], F32, isOutput=False)
    out_d = nc.declare_dram_parameter("out", [S, H], F32, isOutput=True)

    with tile.TileContext(nc) as tc:
        with (
            tc.tile_pool(name="consts", bufs=1) as consts,
            tc.tile_pool(name="u", bufs=2) as upool,
            tc.tile_pool(name="e", bufs=1) as epool,
            tc.tile_pool(name="o", bufs=3) as opool,
            tc.tile_pool(name="psb", bufs=2, space="PSUM") as ps_big,
            tc.tile_pool(name="pss", bufs=1, space="PSUM") as ps_small,
        ):
            # ---- load inputs ----
            xT = consts.tile([H, S], F32)
            nc.sync.dma_start(out=xT, in_=xT_d[:, :])
            xaug = consts.tile([128, 8, XAUG_W], F32)
            for g in range(8):
                nc.sync.dma_start(
                    out=xaug[:, g, :], in_=xaug_d[g * 128 : (g + 1) * 128, :]
                )
            w1ra = consts.tile([H, 128], F32)
            nc.sync.dma_start(out=w1ra, in_=w1ra_d[:, :])
            w1b32 = consts.tile([H, 2, 32], F32)
            nc.sync.dma_start(out=w1b32, in_=w1b_d[:, :, :])
            b1r = consts.tile([128, 1], F32)
            nc.sync.dma_start(out=b1r, in_=b1r_d[:, :])
            w2pad = consts.tile([128, 248], w2bd_dt)
            nc.sync.dma_start(out=w2pad, in_=w2bd_d[:, :])
            mask = consts.tile([128, 128], F32)
            nc.sync.dma_start(out=mask, in_=mask_d[:, :])
            b2r = consts.tile([128, 1], F32)
            nc.sync.dma_start(out=b2r, in_=b2r_d[:, :])

            # ---- FB[p, i] = sum_h W1repA[h, p] * xT[h, i] ----
            FB = consts.tile([128, S], BF16)
            fbp = ps_big.tile([128, S], F32, tag="big")
            for c in range(2):
                sl = slice(c * 512, (c + 1) * 512)
                nc.tensor.matmul(
                    out=fbp[:, sl],
                    lhsT=w1ra[:, :],
                    rhs=xT[:, sl],
                    start=True,
                    stop=True,
                )
            nc.vector.tensor_copy(out=FB, in_=fbp)

            # ---- G[j8*16+a, jb] = sum_h W1b[h, a] * xT[h, 8*jb+j8] + b1[a] ----
            G = consts.tile([128, NBLK], F32)
            gp = ps_big.tile([128, NBLK], F32, tag="big")
            xTg = xT[:, :].rearrange("h (j e) -> h j e", e=8)
            for q in range(4):
                for r in range(2):
                    nc.tensor.matmul(
                        out=gp[32 * q : 32 * (q + 1), :],
                        lhsT=w1b32[:, r, :],
                        rhs=xTg[:, :, 2 * q + r],
                        start=(r == 0),
                        stop=(r == 1),
                        tile_position=(0, 32 * q),
                    )
            nc.vector.tensor_scalar_add(out=G, in0=gp, scalar1=b1r[:, :])

            # ---- out-matmul bookkeeping (interleaved into the main loop;
            # 4 rotating PSUM tiles: ib and ib+4 share tag po{ib%4}) ----
            e_tiles = []
            po_tiles = {}
            next_term = {}  # ib -> next supertile index to accumulate
            active = []

            def activate_ib(ib):
                po_tiles[ib] = ps_small.tile(
                    [128, XAUG_W], F32, tag=f"po{ib % 4}", name=f"po_{ib}"
                )
                next_term[ib] = 0
                active.append(ib)

            def finish_ib(ib):
                po = po_tiles[ib]
                rec = opool.tile([128, 1], F32, tag="rec")
                nc.vector.tensor_scalar_add(
                    out=rec, in0=po[:, H : H + 1], scalar1=1e-10
                )
                nc.vector.reciprocal(out=rec, in_=rec)
                osb = opool.tile([128, H], F32, tag="osb")
                nc.vector.tensor_scalar_mul(out=osb, in0=po[:, 0:H], scalar1=rec)
                nc.sync.dma_start(
                    out=out_d[ib * 128 : (ib + 1) * 128, :], in_=osb
                )
                active.remove(ib)
                if ib + 4 < 8:
                    activate_ib(ib + 4)

            def emit_out_terms(g):
                # out[i,:] = sum_j e[j,i]*x_aug[j]; accumulate terms whose
                # e-supertile is ready, for every ib with a live PSUM slot
                for ib in sorted(active):
                    while next_term[ib] <= min(ib, g):
                        g2 = next_term[ib]
                        col0 = 128 * (ib - g2)
                        nc.tensor.matmul(
                            out=po_tiles[ib][:, :],
                            lhsT=e_tiles[g2][:, col0 : col0 + 128],
                            rhs=xaug[:, g2, :],
                            start=(g2 == 0),
                            stop=(g2 == ib),
                        )
                        next_term[ib] += 1
                    if next_term[ib] > ib:
                        finish_ib(ib)

            for ib in range(4):
                activate_ib(ib)

            # ---- main loop: supertiles of 16 j-blocks (128 j's) ----
            for g in range(8):
                Lg = S - 128 * g  # psum supertile covers columns i in [128g, S)
                ps = ps_big.tile([128, Lg], F32, tag="big")
                # ramp-up: small leading tanh groups so ACT starts early
                group_sizes = [2, 2, 4, 8] if g == 0 else [8, 8]
                done = 0
                for gs in group_sizes:
                    jbs = [16 * g + done + k for k in range(gs)]
                    done += gs
                    offs = []
                    flat = 0
                    for jb in jbs:
                        offs.append(flat)
                        flat += S - 8 * jb
                    U = upool.tile([128, flat], BF16, tag="u")
                    for jb, o in zip(jbs, offs):
                        Lb = S - 8 * jb
                        nc.vector.tensor_scalar_add(
                            out=U[:, o : o + Lb],
                            in0=FB[:, 8 * jb : S],
                            scalar1=G[:, jb : jb + 1],
                        )
                    # tanh output fp16 so score matmuls stream 1 col/cycle
                    TT = upool.tile([128, flat], BF16, tag="tt")
                    nc.scalar.activation(out=TT[:, :], in_=U[:, :], func=FT.Tanh)
                    # score matmuls: M=128 sliding-window block-diag weights
                    # (full-width weights enable fast-weight-load; out base
                    # partition always 0; k=0 zero-inits the whole supertile
                    # because its weight columns outside block 0 are zero)
                    for jb, o in zip(jbs, offs):
                        k = jb - 16 * g  # block index within supertile
                        rel0 = 8 * jb - 128 * g  # == 8k
                        lhs_ap = w2pad[:, 120 - 8 * k : 248 - 8 * k]
                        bounds = (
                            [rel0] + [b for b in (512,) if rel0 < b < Lg] + [Lg]
                        )
                        for c0, c1 in zip(bounds[:-1], bounds[1:]):
                            nc.tensor.matmul(
                                out=ps[:, c0:c1],
                                lhsT=lhs_ap,
                                rhs=TT[:, o + (c0 - rel0) : o + (c1 - rel0)],
                                start=(k == 0),
                                stop=(k == 15),
                            )
                e = epool.tile([128, Lg], F32, tag=f"e{g}")
                nc.scalar.activation(
                    out=e[:, :], in_=ps[:, :], func=FT.Exp, bias=b2r[:, :], scale=1.0
                )
                nc.vector.tensor_mul(e[:, 0:128], e[:, 0:128], mask[:, :])
                e_tiles.append(e)
                emit_out_terms(g)

    nc.compile()
    return nc


_NC_CACHE = None


def _get_nc():
    global _NC_CACHE
    if _NC_CACHE is None:
        _NC_CACHE = _build_nc()
    return _NC_CACHE


def _host_prep(x, W1, b1, w2, b2):
    """Build the per-core input maps (all small derived tensors + shards)."""
    x = np.asarray(x, dtype=np.float32)
    W1 = np.asarray(W1, dtype=np.float32)
    b1 = np.asarray(b1, dtype=np.float32).reshape(-1)
    w2 = np.asarray(w2, dtype=np.float32).reshape(-1)
    b2 = np.asarray(b2, dtype=np.float32).reshape(-1)

    p = np.arange(128)
    W1repA = np.ascontiguousarray(W1[:H][:, p % A])  # [H, 128]
    # W1b32[h, r, m] places g-matmul outputs for j8 = 2q+r at rows 16r+a
    W1b32 = np.zeros((H, 2, 32), dtype=np.float32)
    for r in range(2):
        W1b32[:, r, 16 * r : 16 * r + A] = W1[H:]
    b1rep = np.ascontiguousarray(b1[p % A].reshape(128, 1))
    # sliding-window block-diag weights: W2BDpad[p, 120 + j8] = w2[a]
    # (lhsT for block k is W2BDpad[:, 120-8k : 248-8k])
    W2BDpad = np.zeros((128, 248), dtype=np.float32)
    W2BDpad[p, 120 + p // A] = w2[p % A]
    if SCORE_BF16:
        W2BDpad = W2BDpad.astype(np.float16)
    SUmask = (p[:, None] < p[None, :]).astype(np.float32)  # strictly upper
    b2rep = np.full((128, 1), b2[0], dtype=np.float32)

    shared = {
        "W1repA": W1repA,
        "W1b32": W1b32,
        "b1rep": b1rep,
        "W2BDpad": W2BDpad,
        "SUmask": SUmask,
        "b2rep": b2rep,
    }
    in_maps = []
    for c in range(NCORES):
        xb = x[c]  # [S, H]
        x_aug = np.zeros((S, XAUG_W), dtype=np.float32)
        x_aug[:, :H] = xb
        x_aug[:, H] = 1.0
        m = dict(shared)
        m["x_aug"] = x_aug
        m["xT"] = np.ascontiguousarray(xb.T)
        in_maps.append(m)
    return in_maps


def kernel(x, W1, b1, w2, b2, _trace=False):
    nc = _get_nc()
    in_maps = _host_prep(x, W1, b1, w2, b2)
    res = run_bass_kernel_spmd(nc, in_maps, list(range(NCORES)), trace=_trace)
    out = np.stack([np.asarray(res.results[c]["out"]) for c in range(NCORES)])
    if _trace:
        kernel.last_exec_time_ns = res.exec_time_ns
        kernel.last_profile = res.profile_json
    return out


# revision 27
# speedup vs baseline: 1.1477x; 1.0539x over previous
"""Concatenation (additive/Bahdanau-style) attention Trainium2 kernel.

Math (per batch b):
    f = x @ W1[:H]          # [S, A]
    g = x @ W1[H:]          # [S, A]
    scores[i, j] = w2 . tanh(f[i] + g[j] + b1) + b2
    e = exp(scores) * (j < i)
    out[i] = sum_j e[i, j] x[j] / (sum_j e[i, j] + 1e-10)

Sharding: data-parallel over batch, one batch element per NeuronCore (B=8).
Everything stays on-chip; the [S, S, A] pairwise tensor never exists in HBM.

Per-core layout strategy:
  - j-block = 8 consecutive j values; partitions of the tanh tile hold
    (j8, a) pairs: p = 8*16 grid = j8*16 + a  (8 j's x 16 hidden units).
  - FB[p, i] = f[i, p%16] replicated 8x on partitions (one PE matmul with a
    host-replicated W1a).
  - G[p, jb] = g[8*jb + j8, a] + b1[a] with p = j8*16+a  (8 strided matmuls).
  - U[p, i] = FB[p, i] + G[p, jb]  (DVE tensor_scalar add, per-partition
    scalar) for the exact triangular range i >= 8*jb, then one big ACT tanh
    per group of 8 j-blocks (amortizes ACT fixed overhead).
  - scores via PE matmul with block-diag W2BD[p, m] = (p//16==m) * w2[p%16]
    contracting all 128 partitions -> [8 j, i] rows in PSUM; 16 j-blocks
    fill a [128, Lg] PSUM supertile (j = 128*g + p).
  - one ACT exp (bias=b2) per supertile PSUM -> SBUF e-tile; strictly-upper
    [128,128] mask on the diagonal chunk enforces j < i.
  - out: for each 128-row i-block, accumulate matmuls over supertiles g<=ib:
    lhsT = e_g[:, i-cols] (K=j), rhs = x_aug (x with a ones column) so the
    softmax denominator falls out of the same matmuls; then reciprocal+scale.
"""

import numpy as np

import concourse.bass as bass
import concourse.tile as tile
from concourse import bacc, mybir
from concourse.bass_utils import run_bass_kernel_spmd

B, S, H, A = 8, 1024, 128, 16
NCORES = 8
XAUG_W = H + 4  # x plus a ones column, padded to 132 floats (528 B)
NBLK = S // 8  # 128 j-blocks of 8

FT = mybir.ActivationFunctionType
F32 = mybir.dt.float32
BF16 = mybir.dt.float16  # fp16: same 1 col/cycle as bf16, 8x the mantissa

# Score-matmul dtype knob: bf16 streams 1 col/cycle (vs 4 for float32).
# (float32r also streams 1 col/cycle but requires dst partition 0 and is
# no more precise than bf16 on TRN2.)
SCORE_BF16 = True


def _build_nc():
    nc = bacc.Bacc(None)

    xaug_d = nc.declare_dram_parameter("x_aug", [S, XAUG_W], F32, isOutput=False)
    xT_d = nc.declare_dram_parameter("xT", [H, S], F32, isOutput=False)
    w1ra_d = nc.declare_dram_parameter("W1repA", [H, 128], F32, isOutput=False)
    w1b_d = nc.declare_dram_parameter("W1b32", [H, 2, 32], F32, isOutput=False)
    b1r_d = nc.declare_dram_parameter("b1rep", [128, 1], F32, isOutput=False)
    w2bd_dt = BF16 if SCORE_BF16 else F32
    w2bd_d = nc.declare_dram_parameter("W2BDpad", [128, 248], w2bd_dt, isOutput=False)
    mask_d = nc.declare_dram_parameter("SUmask", [128, 128], F32, isOutput=False)
    b2r_d = nc.declare_dram_parameter("b2rep", [128, 1], F32, isOutput=False)
    out_d = nc.declare_dram_parameter("out", [S, H], F32, isOutput=True)

    with tile.TileContext(nc) as tc:
        with (
            tc.tile_pool(name="consts", bufs=1) as consts,
            tc.tile_pool(name="u", bufs=2) as upool,
            tc.tile_pool(name="e", bufs=1) as epool,
            tc.tile_pool(name="o", bufs=3) as opool,
            tc.tile_pool(name="psb", bufs=2, space="PSUM") as ps_big,
            tc.tile_pool(name="pss", bufs=1, space="PSUM") as ps_small,
        ):
            # ---- load inputs: only SP + ACT have HW DGE queues on TRN2;
            # critical loads split across both, bulk loads on gpsimd SWDGE
            xT = consts.tile([H, S], F32)
            nc.sync.dma_start(out=xT[:, 0:512], in_=xT_d[:, 0:512])
            nc.scalar.dma_start(out=xT[:, 512:S], in_=xT_d[:, 512:S])
            w1ra = consts.tile([H, 128], F32)
            nc.sync.dma_start(out=w1ra, in_=w1ra_d[:, :])
            w1b32 = consts.tile([H, 2, 32], F32)
            nc.scalar.dma_start(out=w1b32, in_=w1b_d[:, :, :])
            b1r = consts.tile([128, 1], F32)
            nc.sync.dma_start(out=b1r, in_=b1r_d[:, :])
            w2pad = consts.tile([128, 248], w2bd_dt)
            nc.scalar.dma_start(out=w2pad, in_=w2bd_d[:, :])

            # warm the PE clock (HAM un-throttles after ~3.4us of sustained
            # work) and preload the exp/tanh ACT table while DMAs run
            scratch = consts.tile([128, 1], F32)
            nc.vector.memset(scratch, 0.0)
            nc.scalar.activation(out=scratch, in_=scratch, func=FT.Tanh)
            wsrc = consts.tile([128, 512], BF16)
            nc.vector.memset(wsrc, 0.0)
            wps = ps_small.tile([128, 512], F32, tag="po0", name="warm_ps")
            for _ in range(10):
                nc.tensor.matmul(
                    out=wps[:, :],
                    lhsT=wsrc[:, 0:128],
                    rhs=wsrc[:, :],
                    start=True,
                    stop=True,
                )

            xaug = consts.tile([128, 8, XAUG_W], F32)
            nc.gpsimd.dma_start(
                out=xaug[:, :, :],
                in_=xaug_d[:, :].rearrange("(g p) w -> p g w", p=128),
            )
            mask = consts.tile([128, 128], F32)
            nc.gpsimd.dma_start(out=mask, in_=mask_d[:, :])
            b2r = consts.tile([128, 1], F32)
            nc.gpsimd.dma_start(out=b2r, in_=b2r_d[:, :])

            # ---- FB[p, i] = sum_h W1repA[h, p] * xT[h, i] ----
            FB = consts.tile([128, S], BF16)
            fbp = ps_big.tile([128, S], F32, tag="big")
            for c in range(2):
                sl = slice(c * 512, (c + 1) * 512)
                nc.tensor.matmul(
                    out=fbp[:, sl],
                    lhsT=w1ra[:, :],
                    rhs=xT[:, sl],
                    start=True,
                    stop=True,
                )
            nc.vector.tensor_copy(out=FB[:, 0:512], in_=fbp[:, 0:512])
            nc.vector.tensor_copy(out=FB[:, 512:S], in_=fbp[:, 512:S])

            # ---- G[j8*16+a, jb] = sum_h W1b[h, a] * xT[h, 8*jb+j8] + b1[a] ----
            G = consts.tile([128, NBLK], F32)
            gp = ps_big.tile([128, NBLK], F32, tag="big")
            xTg = xT[:, :].rearrange("h (j e) -> h j e", e=8)
            for q in range(4):
                for r in range(2):
                    nc.tensor.matmul(
                        out=gp[32 * q : 32 * (q + 1), :],
                        lhsT=w1b32[:, r, :],
                        rhs=xTg[:, :, 2 * q + r],
                        start=(r == 0),
                        stop=(r == 1),
                        tile_position=(0, 32 * q),
                    )
            nc.vector.tensor_scalar_add(out=G, in0=gp, scalar1=b1r[:, :])

            # ---- out-matmul bookkeeping (interleaved into the main loop;
            # 4 rotating PSUM tiles: ib and ib+4 share tag po{ib%4}) ----
            e_tiles = []
            po_tiles = {}
            next_term = {}  # ib -> next supertile index to accumulate
            active = []

            def activate_ib(ib):
                po_tiles[ib] = ps_small.tile(
                    [128, XAUG_W], F32, tag=f"po{ib % 4}", name=f"po_{ib}"
                )
                next_term[ib] = 0
                active.append(ib)

            def finish_ib(ib):
                po = po_tiles[ib]
                rec = opool.tile([128, 1], F32, tag="rec")
                nc.vector.tensor_scalar_add(
                    out=rec, in0=po[:, H : H + 1], scalar1=1e-10
                )
                nc.vector.reciprocal(out=rec, in_=rec)
                osb = opool.tile([128, H], F32, tag="osb")
                nc.vector.tensor_scalar_mul(out=osb, in0=po[:, 0:H], scalar1=rec)
                nc.sync.dma_start(
                    out=out_d[ib * 128 : (ib + 1) * 128, :], in_=osb
                )
                active.remove(ib)
                if ib + 4 < 8:
                    activate_ib(ib + 4)

            def emit_out_terms(g):
                # out[i,:] = sum_j e[j,i]*x_aug[j]; accumulate terms whose
                # e-supertile is ready, for every ib with a live PSUM slot
                for ib in sorted(active):
                    while next_term[ib] <= min(ib, g):
                        g2 = next_term[ib]
                        col0 = 128 * (ib - g2)
                        nc.tensor.matmul(
                            out=po_tiles[ib][:, :],
                            lhsT=e_tiles[g2][:, col0 : col0 + 128],
                            rhs=xaug[:, g2, :],
                            start=(g2 == 0),
                            stop=(g2 == ib),
                        )
                        next_term[ib] += 1
                    if next_term[ib] > ib:
                        finish_ib(ib)

            for ib in range(4):
                activate_ib(ib)

            # ---- main loop: supertiles of 16 j-blocks (128 j's) ----
            for g in range(8):
                Lg = S - 128 * g  # psum supertile covers columns i in [128g, S)
                ps = ps_big.tile([128, Lg], F32, tag="big")
                # ramp-up: small leading tanh groups so ACT starts early
                group_sizes = [2, 2, 4, 8] if g == 0 else [8, 8]
                done = 0
                for gs in group_sizes:
                    jbs = [16 * g + done + k for k in range(gs)]
                    done += gs
                    offs = []
                    flat = 0
                    for jb in jbs:
                        offs.append(flat)
                        flat += S - 8 * jb
                    U = upool.tile([128, flat], BF16, tag="u")
                    for jb, o in zip(jbs, offs):
                        Lb = S - 8 * jb
                        nc.vector.tensor_scalar_add(
                            out=U[:, o : o + Lb],
                            in0=FB[:, 8 * jb : S],
                            scalar1=G[:, jb : jb + 1],
                        )
                    # tanh output fp16 so score matmuls stream 1 col/cycle
                    TT = upool.tile([128, flat], BF16, tag="tt")
                    nc.scalar.activation(out=TT[:, :], in_=U[:, :], func=FT.Tanh)
                    # score matmuls: M=128 sliding-window block-diag weights
                    # (full-width weights enable fast-weight-load; out base
                    # partition always 0; k=0 zero-inits the whole supertile
                    # because its weight columns outside block 0 are zero)
                    for jb, o in zip(jbs, offs):
                        k = jb - 16 * g  # block index within supertile
                        rel0 = 8 * jb - 128 * g  # == 8k
                        lhs_ap = w2pad[:, 120 - 8 * k : 248 - 8 * k]
                        bounds = (
                            [rel0] + [b for b in (512,) if rel0 < b < Lg] + [Lg]
                        )
                        for c0, c1 in zip(bounds[:-1], bounds[1:]):
                            nc.tensor.matmul(
                                out=ps[:, c0:c1],
                                lhsT=lhs_ap,
                                rhs=TT[:, o + (c0 - rel0) : o + (c1 - rel0)],
                                start=(k == 0),
                                stop=(k == 15),
                            )
                e = epool.tile([128, Lg], F32, tag=f"e{g}")
                nc.scalar.activation(
                    out=e[:, :], in_=ps[:, :], func=FT.Exp, bias=b2r[:, :], scale=1.0
                )
                nc.vector.tensor_mul(e[:, 0:128], e[:, 0:128], mask[:, :])
                e_tiles.append(e)
                emit_out_terms(g)

    nc.compile()
    return nc


_NC_CACHE = None


def _get_nc():
    global _NC_CACHE
    if _NC_CACHE is None:
        _NC_CACHE = _build_nc()
    return _NC_CACHE


def _host_prep(x, W1, b1, w2, b2):
    """Build the per-core input maps (all small derived tensors + shards)."""
    x = np.asarray(x, dtype=np.float32)
    W1 = np.asarray(W1, dtype=np.float32)
    b1 = np.asarray(b1, dtype=np.float32).reshape(-1)
    w2 = np.asarray(w2, dtype=np.float32).reshape(-1)
    b2 = np.asarray(b2, dtype=np.float32).reshape(-1)

    p = np.arange(128)
    W1repA = np.ascontiguousarray(W1[:H][:, p % A])  # [H, 128]
    # W1b32[h, r, m] places g-matmul outputs for j8 = 2q+r at rows 16r+a
    W1b32 = np.zeros((H, 2, 32), dtype=np.float32)
    for r in range(2):
        W1b32[:, r, 16 * r : 16 * r + A] = W1[H:]
    b1rep = np.ascontiguousarray(b1[p % A].reshape(128, 1))
    # sliding-window block-diag weights: W2BDpad[p, 120 + j8] = w2[a]
    # (lhsT for block k is W2BDpad[:, 120-8k : 248-8k])
    W2BDpad = np.zeros((128, 248), dtype=np.float32)
    W2BDpad[p, 120 + p // A] = w2[p % A]
    if SCORE_BF16:
        W2BDpad = W2BDpad.astype(np.float16)
    SUmask = (p[:, None] < p[None, :]).astype(np.float32)  # strictly upper
    b2rep = np.full((128, 1), b2[0], dtype=np.float32)

    shared = {
        "W1repA": W1repA,
        "W1b32": W1b32,
        "b1rep": b1rep,
        "W2BDpad": W2BDpad,
        "SUmask": SUmask,
        "b2rep": b2rep,
    }
    in_maps = []
    for c in range(NCORES):
        xb = x[c]  # [S, H]
        x_aug = np.zeros((S, XAUG_W), dtype=np.float32)
        x_aug[:, :H] = xb
        x_aug[:, H] = 1.0
        m = dict(shared)
        m["x_aug"] = x_aug
        m["xT"] = np.ascontiguousarray(xb.T)
        in_maps.append(m)
    return in_maps


def kernel(x, W1, b1, w2, b2, _trace=False):
    nc = _get_nc()
    in_maps = _host_prep(x, W1, b1, w2, b2)
    res = run_bass_kernel_spmd(nc, in_maps, list(range(NCORES)), trace=_trace)
    out = np.stack([np.asarray(res.results[c]["out"]) for c in range(NCORES)])
    if _trace:
        kernel.last_exec_time_ns = res.exec_time_ns
        kernel.last_profile = res.profile_json
    return out


# revision 28
# speedup vs baseline: 1.1728x; 1.0219x over previous
"""Concatenation (additive/Bahdanau-style) attention Trainium2 kernel.

Math (per batch b):
    f = x @ W1[:H]          # [S, A]
    g = x @ W1[H:]          # [S, A]
    scores[i, j] = w2 . tanh(f[i] + g[j] + b1) + b2
    e = exp(scores) * (j < i)
    out[i] = sum_j e[i, j] x[j] / (sum_j e[i, j] + 1e-10)

Sharding: data-parallel over batch, one batch element per NeuronCore (B=8).
Everything stays on-chip; the [S, S, A] pairwise tensor never exists in HBM.

Per-core layout strategy:
  - j-block = 8 consecutive j values; partitions of the tanh tile hold
    (j8, a) pairs: p = 8*16 grid = j8*16 + a  (8 j's x 16 hidden units).
  - FB[p, i] = f[i, p%16] replicated 8x on partitions (one PE matmul with a
    host-replicated W1a).
  - G[p, jb] = g[8*jb + j8, a] + b1[a] with p = j8*16+a  (8 strided matmuls).
  - U[p, i] = FB[p, i] + G[p, jb]  (DVE tensor_scalar add, per-partition
    scalar) for the exact triangular range i >= 8*jb, then one big ACT tanh
    per group of 8 j-blocks (amortizes ACT fixed overhead).
  - scores via PE matmul with block-diag W2BD[p, m] = (p//16==m) * w2[p%16]
    contracting all 128 partitions -> [8 j, i] rows in PSUM; 16 j-blocks
    fill a [128, Lg] PSUM supertile (j = 128*g + p).
  - one ACT exp (bias=b2) per supertile PSUM -> SBUF e-tile; strictly-upper
    [128,128] mask on the diagonal chunk enforces j < i.
  - out: for each 128-row i-block, accumulate matmuls over supertiles g<=ib:
    lhsT = e_g[:, i-cols] (K=j), rhs = x_aug (x with a ones column) so the
    softmax denominator falls out of the same matmuls; then reciprocal+scale.
"""

import numpy as np

import concourse.bass as bass
import concourse.tile as tile
from concourse import bacc, mybir
from concourse.bass_utils import run_bass_kernel_spmd

B, S, H, A = 8, 1024, 128, 16
NCORES = 8
XAUG_W = H + 4  # x plus a ones column, padded to 132 floats (528 B)
NBLK = S // 8  # 128 j-blocks of 8

FT = mybir.ActivationFunctionType
F32 = mybir.dt.float32
BF16 = mybir.dt.float16  # fp16: same 1 col/cycle as bf16, 8x the mantissa

# Score-matmul dtype knob: bf16 streams 1 col/cycle (vs 4 for float32).
# (float32r also streams 1 col/cycle but requires dst partition 0 and is
# no more precise than bf16 on TRN2.)
SCORE_BF16 = True


def _build_nc():
    nc = bacc.Bacc(None)

    xaug_d = nc.declare_dram_parameter("x_aug", [S, XAUG_W], F32, isOutput=False)
    xT_d = nc.declare_dram_parameter("xT", [H, S], BF16, isOutput=False)
    w1ra_d = nc.declare_dram_parameter("W1repA", [H, 128], BF16, isOutput=False)
    w1b_d = nc.declare_dram_parameter("W1b32", [H, 2, 32], BF16, isOutput=False)
    b1r_d = nc.declare_dram_parameter("b1rep", [128, 1], F32, isOutput=False)
    w2bd_dt = BF16 if SCORE_BF16 else F32
    w2bd_d = nc.declare_dram_parameter("W2BDpad", [128, 248], w2bd_dt, isOutput=False)
    mask_d = nc.declare_dram_parameter("SUmask", [128, 128], F32, isOutput=False)
    b2r_d = nc.declare_dram_parameter("b2rep", [128, 1], F32, isOutput=False)
    out_d = nc.declare_dram_parameter("out", [S, H], F32, isOutput=True)

    with tile.TileContext(nc) as tc:
        with (
            tc.tile_pool(name="consts", bufs=1) as consts,
            tc.tile_pool(name="u", bufs=2) as upool,
            tc.tile_pool(name="e", bufs=1) as epool,
            tc.tile_pool(name="o", bufs=3) as opool,
            tc.tile_pool(name="psb", bufs=2, space="PSUM") as ps_big,
            tc.tile_pool(name="pss", bufs=1, space="PSUM") as ps_small,
        ):
            # ---- load inputs: only SP + ACT have HW DGE queues on TRN2;
            # critical loads split across both, bulk loads on gpsimd SWDGE
            xT = consts.tile([H, S], BF16)
            nc.sync.dma_start(out=xT[:, 0:512], in_=xT_d[:, 0:512])
            nc.scalar.dma_start(out=xT[:, 512:S], in_=xT_d[:, 512:S])
            w1ra = consts.tile([H, 128], BF16)
            nc.sync.dma_start(out=w1ra, in_=w1ra_d[:, :])
            w1b32 = consts.tile([H, 2, 32], BF16)
            nc.scalar.dma_start(out=w1b32, in_=w1b_d[:, :, :])
            b1r = consts.tile([128, 1], F32)
            nc.sync.dma_start(out=b1r, in_=b1r_d[:, :])
            w2pad = consts.tile([128, 248], w2bd_dt)
            nc.scalar.dma_start(out=w2pad, in_=w2bd_d[:, :])

            # warm the PE clock (HAM un-throttles after ~3.4us of sustained
            # work) and preload the exp/tanh ACT table while DMAs run
            scratch = consts.tile([128, 1], F32)
            nc.vector.memset(scratch, 0.0)
            nc.scalar.activation(out=scratch, in_=scratch, func=FT.Tanh)
            wsrc = consts.tile([128, 512], BF16)
            nc.vector.memset(wsrc, 0.0)
            wps = ps_small.tile([128, 512], F32, tag="po0", name="warm_ps")
            for _ in range(8):
                nc.tensor.matmul(
                    out=wps[:, :],
                    lhsT=wsrc[:, 0:128],
                    rhs=wsrc[:, :],
                    start=True,
                    stop=True,
                )

            xaug = consts.tile([128, 8, XAUG_W], F32)
            nc.gpsimd.dma_start(
                out=xaug[:, :, :],
                in_=xaug_d[:, :].rearrange("(g p) w -> p g w", p=128),
            )
            mask = consts.tile([128, 128], F32)
            nc.gpsimd.dma_start(out=mask, in_=mask_d[:, :])
            b2r = consts.tile([128, 1], F32)
            nc.gpsimd.dma_start(out=b2r, in_=b2r_d[:, :])

            # ---- FB[p, i] = sum_h W1repA[h, p] * xT[h, i] ----
            FB = consts.tile([128, S], BF16)
            fbp = ps_big.tile([128, S], F32, tag="big")
            for c in range(2):
                sl = slice(c * 512, (c + 1) * 512)
                nc.tensor.matmul(
                    out=fbp[:, sl],
                    lhsT=w1ra[:, :],
                    rhs=xT[:, sl],
                    start=True,
                    stop=True,
                )
            nc.vector.tensor_copy(out=FB[:, 0:512], in_=fbp[:, 0:512])
            nc.vector.tensor_copy(out=FB[:, 512:S], in_=fbp[:, 512:S])

            # ---- G[j8*16+a, jb] = sum_h W1b[h, a] * xT[h, 8*jb+j8] + b1[a] ----
            G = consts.tile([128, NBLK], F32)
            gp = ps_big.tile([128, NBLK], F32, tag="big")
            xTg = xT[:, :].rearrange("h (j e) -> h j e", e=8)
            for q in range(4):
                for r in range(2):
                    nc.tensor.matmul(
                        out=gp[32 * q : 32 * (q + 1), :],
                        lhsT=w1b32[:, r, :],
                        rhs=xTg[:, :, 2 * q + r],
                        start=(r == 0),
                        stop=(r == 1),
                        tile_position=(0, 32 * q),
                    )
            nc.vector.tensor_scalar_add(out=G, in0=gp, scalar1=b1r[:, :])

            # ---- out-matmul bookkeeping (interleaved into the main loop;
            # 4 rotating PSUM tiles: ib and ib+4 share tag po{ib%4}) ----
            e_tiles = []
            po_tiles = {}
            next_term = {}  # ib -> next supertile index to accumulate
            active = []

            def activate_ib(ib):
                po_tiles[ib] = ps_small.tile(
                    [128, XAUG_W], F32, tag=f"po{ib % 4}", name=f"po_{ib}"
                )
                next_term[ib] = 0
                active.append(ib)

            def finish_ib(ib):
                po = po_tiles[ib]
                rec = opool.tile([128, 1], F32, tag="rec")
                nc.vector.tensor_scalar_add(
                    out=rec, in0=po[:, H : H + 1], scalar1=1e-10
                )
                nc.vector.reciprocal(out=rec, in_=rec)
                osb = opool.tile([128, H], F32, tag="osb")
                nc.vector.tensor_scalar_mul(out=osb, in0=po[:, 0:H], scalar1=rec)
                nc.sync.dma_start(
                    out=out_d[ib * 128 : (ib + 1) * 128, :], in_=osb
                )
                active.remove(ib)
                if ib + 4 < 8:
                    activate_ib(ib + 4)

            def emit_out_terms(g):
                # out[i,:] = sum_j e[j,i]*x_aug[j]; accumulate terms whose
                # e-supertile is ready, for every ib with a live PSUM slot
                for ib in sorted(active):
                    while next_term[ib] <= min(ib, g):
                        g2 = next_term[ib]
                        col0 = 128 * (ib - g2)
                        nc.tensor.matmul(
                            out=po_tiles[ib][:, :],
                            lhsT=e_tiles[g2][:, col0 : col0 + 128],
                            rhs=xaug[:, g2, :],
                            start=(g2 == 0),
                            stop=(g2 == ib),
                        )
                        next_term[ib] += 1
                    if next_term[ib] > ib:
                        finish_ib(ib)

            for ib in range(4):
                activate_ib(ib)

            # ---- main loop: supertiles of 16 j-blocks (128 j's) ----
            for g in range(8):
                Lg = S - 128 * g  # psum supertile covers columns i in [128g, S)
                ps = ps_big.tile([128, Lg], F32, tag="big")
                # ramp-up: small leading tanh groups so ACT starts early
                group_sizes = [1, 1, 2, 4, 8] if g == 0 else [8, 8]
                done = 0
                for gs in group_sizes:
                    jbs = [16 * g + done + k for k in range(gs)]
                    done += gs
                    offs = []
                    flat = 0
                    for jb in jbs:
                        offs.append(flat)
                        flat += S - 8 * jb
                    U = upool.tile([128, flat], BF16, tag="u")
                    for jb, o in zip(jbs, offs):
                        Lb = S - 8 * jb
                        nc.vector.tensor_scalar_add(
                            out=U[:, o : o + Lb],
                            in0=FB[:, 8 * jb : S],
                            scalar1=G[:, jb : jb + 1],
                        )
                    # tanh output fp16 so score matmuls stream 1 col/cycle
                    TT = upool.tile([128, flat], BF16, tag="tt")
                    nc.scalar.activation(out=TT[:, :], in_=U[:, :], func=FT.Tanh)
                    # score matmuls: M=128 sliding-window block-diag weights
                    # (full-width weights enable fast-weight-load; out base
                    # partition always 0; k=0 zero-inits the whole supertile
                    # because its weight columns outside block 0 are zero)
                    for jb, o in zip(jbs, offs):
                        k = jb - 16 * g  # block index within supertile
                        rel0 = 8 * jb - 128 * g  # == 8k
                        lhs_ap = w2pad[:, 120 - 8 * k : 248 - 8 * k]
                        bounds = (
                            [rel0] + [b for b in (512,) if rel0 < b < Lg] + [Lg]
                        )
                        for c0, c1 in zip(bounds[:-1], bounds[1:]):
                            nc.tensor.matmul(
                                out=ps[:, c0:c1],
                                lhsT=lhs_ap,
                                rhs=TT[:, o + (c0 - rel0) : o + (c1 - rel0)],
                                start=(k == 0),
                                stop=(k == 15),
                            )
                e = epool.tile([128, Lg], F32, tag=f"e{g}")
                nc.scalar.activation(
                    out=e[:, :], in_=ps[:, :], func=FT.Exp, bias=b2r[:, :], scale=1.0
                )
                nc.vector.tensor_mul(e[:, 0:128], e[:, 0:128], mask[:, :])
                e_tiles.append(e)
                emit_out_terms(g)

    nc.compile()
    return nc


_NC_CACHE = None


def _get_nc():
    global _NC_CACHE
    if _NC_CACHE is None:
        _NC_CACHE = _build_nc()
    return _NC_CACHE


def _host_prep(x, W1, b1, w2, b2):
    """Build the per-core input maps (all small derived tensors + shards)."""
    x = np.asarray(x, dtype=np.float32)
    W1 = np.asarray(W1, dtype=np.float32)
    b1 = np.asarray(b1, dtype=np.float32).reshape(-1)
    w2 = np.asarray(w2, dtype=np.float32).reshape(-1)
    b2 = np.asarray(b2, dtype=np.float32).reshape(-1)

    p = np.arange(128)
    W1repA = np.ascontiguousarray(W1[:H][:, p % A]).astype(np.float16)  # [H, 128]
    # W1b32[h, r, m] places g-matmul outputs for j8 = 2q+r at rows 16r+a
    W1b32 = np.zeros((H, 2, 32), dtype=np.float16)
    for r in range(2):
        W1b32[:, r, 16 * r : 16 * r + A] = W1[H:]
    b1rep = np.ascontiguousarray(b1[p % A].reshape(128, 1))
    # sliding-window block-diag weights: W2BDpad[p, 120 + j8] = w2[a]
    # (lhsT for block k is W2BDpad[:, 120-8k : 248-8k])
    W2BDpad = np.zeros((128, 248), dtype=np.float32)
    W2BDpad[p, 120 + p // A] = w2[p % A]
    if SCORE_BF16:
        W2BDpad = W2BDpad.astype(np.float16)
    SUmask = (p[:, None] < p[None, :]).astype(np.float32)  # strictly upper
    b2rep = np.full((128, 1), b2[0], dtype=np.float32)

    shared = {
        "W1repA": W1repA,
        "W1b32": W1b32,
        "b1rep": b1rep,
        "W2BDpad": W2BDpad,
        "SUmask": SUmask,
        "b2rep": b2rep,
    }
    in_maps = []
    for c in range(NCORES):
        xb = x[c]  # [S, H]
        x_aug = np.zeros((S, XAUG_W), dtype=np.float32)
        x_aug[:, :H] = xb
        x_aug[:, H] = 1.0
        m = dict(shared)
        m["x_aug"] = x_aug
        m["xT"] = np.ascontiguousarray(xb.T).astype(np.float16)
        in_maps.append(m)
    return in_maps


def kernel(x, W1, b1, w2, b2, _trace=False):
    nc = _get_nc()
    in_maps = _host_prep(x, W1, b1, w2, b2)
    res = run_bass_kernel_spmd(nc, in_maps, list(range(NCORES)), trace=_trace)
    out = np.stack([np.asarray(res.results[c]["out"]) for c in range(NCORES)])
    if _trace:
        kernel.last_exec_time_ns = res.exec_time_ns
        kernel.last_profile = res.profile_json
    return out
